# revision 41
# baseline (speedup 1.0000x reference)
"""3-layer GCN encoder on 8 Trainium2 NeuronCores (Bass/Tile).

Math (per GCNConv, PyG-style, self-loops included):
    out = D^{-1/2} (A + I) D^{-1/2} (x W) + b
Network: relu(conv1(x)) -> relu(conv2(.)) -> h @ W3 + b3.

Distribution: nodes are sharded contiguously across 8 cores (dst-sharded
edges so aggregation is local); activations (p = h_prev @ W) are
AllGather'ed before each conv's message passing.

Per-core static structure (identical program on all cores; only input
data differs):
  - each core owns NPC nodes, assigned to G groups of <=128 "slots"
    (a per-core node permutation balances edges per group and linearizes
    the per-group cumulative degree so edge tiles fit static windows)
  - edges sorted by (group, slot), packed into T tiles of 128 edges per
    group; tile t covers slot window [lo[t], lo[t]+W)
  - gather: one indirect DMA per group fetches the 128*T source rows of
    p_full (256B each)
  - aggregation: per edge tile, matmul(lhsT=msg[128e,64f],
    rhs=S[128e,W]) accumulating into PSUM [64f, 128slots]; S holds the
    edge norm value at the edge's dst slot column (built on DVE from
    iota/is_equal/mult), so scaling is free
  - per-feature bias + relu fused into the PSUM->SBUF activation;
    h stays feature-major [64, slots] which feeds the next GEMM's lhsT
"""

import math
import os
import sys

import numpy as np

for _p in ("/opt/trn_rl_repo",):
    if _p not in sys.path and os.path.isdir(_p):
        sys.path.insert(0, _p)

import concourse.bass as bass
import concourse.mybir as mybir
import concourse.tile as tile
from concourse.bacc import Bacc
from concourse.bass import IndirectOffsetOnAxis
from concourse.bass_utils import run_bass_kernel_spmd

F32 = mybir.dt.float32
I32 = mybir.dt.int32
P = 128  # partitions / edge-tile size
FEAT = 64  # in/hid/out channels (all 64 for this problem)
MSTR = FEAT + 16  # msg tile stride: pad so the gather dest AP stays 3D
# (walrus merges contiguous AP dims, then emits one DMA descriptor per
# OUTER dest dim — a merged 2D dest collapses the per-edge gather into one
# contiguous stream per partition)

N_NODES = 100000
N_CORES = 8


# --------------------------------------------------------------------------
# Host-side preprocessing: node permutation, edge tiling, input arrays
# --------------------------------------------------------------------------


def _assign_groups_and_slots(deg, n_cores, npc, G):
    """Per core: snake-deal nodes (by degree desc) into G groups, then order
    each group's nodes so cumulative degree grows ~linearly slot by slot.
    Returns node_group[N], node_slot[N] (int32)."""
    N = deg.shape[0]
    node_group = np.empty(N, np.int32)
    node_slot = np.empty(N, np.int32)
    for c in range(n_cores):
        lo, hi = c * npc, (c + 1) * npc
        nodes = np.arange(lo, hi)
        d = deg[lo:hi]
        order = np.argsort(-d, kind="stable")
        # snake deal across groups -> balanced per-group edge totals
        i = np.arange(npc)
        rounds = i // G
        pos = i % G
        grp = np.where(rounds % 2 == 0, pos, G - 1 - pos)
        node_group[nodes[order]] = grp
        # per-group greedy slot ordering to linearize cumsum(degree)
        for g in range(G):
            members = nodes[order[grp == g]]
            degs = deg[members]
            # sort ascending; greedily pick element moving cumsum closest
            # to the linear target
            srt = np.argsort(degs, kind="stable")
            rem_deg = list(degs[srt])
            rem_node = list(members[srt])
            total = float(degs.sum())
            L = len(members)
            cum = 0.0
            import bisect

            for s in range(L):
                target = total * (s + 1) / L
                want = target - cum
                j = bisect.bisect_left(rem_deg, want)
                if j >= len(rem_deg):
                    j = len(rem_deg) - 1
                elif j > 0:
                    # pick closer of j, j-1
                    if abs(rem_deg[j] - want) >= abs(rem_deg[j - 1] - want):
                        j = j - 1
                cum += rem_deg[j]
                node_slot[rem_node[j]] = s
                del rem_deg[j], rem_node[j]
    return node_group, node_slot


def _place_edges(s_arr, T, lo, W):
    """Greedy placement of (slot-sorted) edges of one group into T tiles of
    128, honoring static windows. Returns (t_idx, p_idx) arrays or raises."""
    n = len(s_arr)
    t_idx = np.empty(n, np.int32)
    p_idx = np.empty(n, np.int32)
    idx = 0
    for t in range(T):
        if idx >= n:
            break
        hi = np.searchsorted(s_arr, lo[t] + W)  # edges with slot < lo[t]+W
        take = min(P, hi - idx)
        if take > 0:
            if s_arr[idx] < lo[t]:
                raise RuntimeError("edge behind window")
            t_idx[idx : idx + take] = t
            p_idx[idx : idx + take] = np.arange(take)
            idx += take
    if idx < n:
        raise RuntimeError("edges overflow T tiles")
    return t_idx, p_idx


def preprocess(x, edge_index, n_cores=N_CORES):
    """Build all per-core device input arrays + output unpermute map."""
    x = np.asarray(x, np.float32)
    N = x.shape[0]
    npc = N // n_cores
    assert npc * n_cores == N
    G = math.ceil(npc / P)
    SLOTS = G * P

    src = np.concatenate([np.asarray(edge_index[0]), np.arange(N)]).astype(np.int64)
    dst = np.concatenate([np.asarray(edge_index[1]), np.arange(N)]).astype(np.int64)
    deg = np.bincount(dst, minlength=N)
    dinv = (np.float32(1.0) / np.sqrt(deg.astype(np.float32))).astype(np.float32)
    norm = (dinv[src] * dinv[dst]).astype(np.float32)

    node_group, node_slot = _assign_groups_and_slots(deg, n_cores, npc, G)
    core_of_node = (np.arange(N) // npc).astype(np.int64)
    node_row = (core_of_node * SLOTS + node_group * P + node_slot).astype(np.int32)

    e_core = (dst // npc).astype(np.int32)
    e_grp = node_group[dst]
    e_slot = node_slot[dst]

    # T = max tiles needed over all (core, group)
    key = (e_core.astype(np.int64) * G + e_grp).astype(np.int64)
    counts = np.bincount(key, minlength=n_cores * G)
    T = int(math.ceil(counts.max() / P))

    placed = None
    for W in (16, 24, 32, 48, 64, 96, 128):
        lo = np.clip(np.round(P * (np.arange(T) + 0.5) / T - W / 2), 0, P - W).astype(
            np.int32
        )
        try:
            # sort edges by (core, group, slot)
            order = np.lexsort((e_slot, e_grp, e_core))
            so_src, so_norm = src[order], norm[order]
            so_core, so_grp, so_slot = e_core[order], e_grp[order], e_slot[order]
            t_all = np.empty(len(order), np.int32)
            p_all = np.empty(len(order), np.int32)
            starts = np.searchsorted(
                so_core.astype(np.int64) * G + so_grp, np.arange(n_cores * G)
            )
            ends = np.append(starts[1:], len(order))
            for k in range(n_cores * G):
                a, b = starts[k], ends[k]
                if a == b:
                    continue
                t_idx, p_idx = _place_edges(so_slot[a:b], T, lo, W)
                t_all[a:b], p_all[a:b] = t_idx, p_idx
            placed = (order, so_src, so_norm, so_core, so_grp, so_slot, t_all, p_all)
            break
        except RuntimeError:
            continue
    if placed is None:
        raise RuntimeError("could not fit edges into static windows")
    (order, so_src, so_norm, so_core, so_grp, so_slot, t_all, p_all) = placed

    cols = so_grp * T + t_all  # column in [G*T)
    per_core = []
    for c in range(n_cores):
        m = so_core == c
        srcs = np.zeros((P, G * T), np.int32)
        drel = np.full((P, G * T), -1.0, np.float32)
        nrm = np.zeros((P, G * T), np.float32)
        pc, cc = p_all[m], cols[m]
        srcs[pc, cc] = node_row[so_src[m]]
        drel[pc, cc] = (so_slot[m] - lo[t_all[m]]).astype(np.float32)
        nrm[pc, cc] = so_norm[m]
        # xT shard: [64, SLOTS]
        nodes = np.arange(c * npc, (c + 1) * npc)
        xT = np.zeros((SLOTS, FEAT), np.float32)
        xT[node_group[nodes] * P + node_slot[nodes]] = x[nodes]
        xT = np.ascontiguousarray(xT.T)
        per_core.append(dict(srcs=srcs, drel=drel, nrm=nrm, xT=xT))

    iota = np.tile(np.arange(W, dtype=np.float32), T)[None, :].repeat(P, 0).copy()
    meta = dict(
        n_cores=n_cores, npc=npc, G=G, T=T, W=W, SLOTS=SLOTS, lo=lo, node_row=node_row
    )
    return per_core, iota, meta


# --------------------------------------------------------------------------
# Device program
# --------------------------------------------------------------------------


def blob_layout(meta):
    """Column layout of the [128, CB] constant blob (everything except
    xT+W1, which ride in their own tensor for the layer-1 GEMM's single
    DMA-lane wait)."""
    G, T, W = meta["G"], meta["T"], meta["W"]
    off = {}
    c = 0
    for name, width in (
        ("b3b", FEAT),
        ("iota", T * W),
        ("drel", G * T),
        ("nrm", G * T),
        ("srcs", G * T),
        ("W2", FEAT),
        ("W3", FEAT),
        ("b1", 1),
        ("b2", 1),
    ):
        off[name] = (c, width)
        c += width
    return off, c


def build_nc(meta, debug_outputs=False):
    C, G, T, W, SLOTS = (
        meta["n_cores"],
        meta["G"],
        meta["T"],
        meta["W"],
        meta["SLOTS"],
    )
    lo = meta["lo"]
    off, CB = blob_layout(meta)
    # Bacc (not raw Bass): its compile() runs generate_event_semaphores,
    # which splits multi-wait instructions into EventSemaphore ops to meet
    # the TRN2 one-wait-per-instruction constraint.
    nc = Bacc(num_devices=C)

    # fp32 matmuls tolerate only ONE sync wait in walrus codegen, so inputs
    # are packed to give every matmul at most one unseen producer.
    xtw_h = nc.dram_tensor("xtw", [FEAT, SLOTS + FEAT], F32, kind="ExternalInput")
    blob_h = nc.dram_tensor("blob", [P, CB], F32, kind="ExternalInput")
    out_h = nc.dram_tensor("out", [SLOTS, FEAT], F32, kind="ExternalOutput")
    if debug_outputs:
        dbg_p1l = nc.dram_tensor("dbg_p1l", [SLOTS, FEAT], F32, kind="ExternalOutput")
        dbg_p1f = nc.dram_tensor(
            "dbg_p1f", [C * SLOTS, FEAT], F32, kind="ExternalOutput"
        )
        dbg_h1 = nc.dram_tensor("dbg_h1", [FEAT, SLOTS], F32, kind="ExternalOutput")
        dbg_msg = nc.dram_tensor("dbg_msg", [P, T * FEAT], F32, kind="ExternalOutput")
        dbg_s = nc.dram_tensor("dbg_s", [P, T * W], F32, kind="ExternalOutput")

    with tile.TileContext(nc) as tc, tc.tile_pool(name="consts", bufs=1) as consts:
        # ---- persistent SBUF ----
        h1_sb = consts.tile([FEAT, SLOTS], F32, name="h1_sb")
        blob_sb = consts.tile([P, CB], F32, name="blob_sb")
        W2_sb = consts.tile([FEAT, FEAT], F32, name="W2_sb")
        W3_sb = consts.tile([FEAT, FEAT], F32, name="W3_sb")

        nc.sync.dma_start(out=blob_sb[:], in_=blob_h[:])

        def blob(name):
            c, w = off[name]
            return blob_sb[:, c : c + w]

        def blob64(name):
            c, w = off[name]
            return blob_sb[0:FEAT, c : c + w]

        # PE-read constants go through DVE so later GEMMs wait on DVE only
        nc.vector.tensor_copy(out=W2_sb[:], in_=blob64("W2"))
        nc.vector.tensor_copy(out=W3_sb[:], in_=blob64("W3"))

        b1_sb = blob64("b1")
        b2_sb = blob64("b2")
        b3b_sb = blob("b3b")
        iota_sb = blob("iota")
        drel_sb = blob("drel")
        nrm_sb = blob("nrm")
        srcs_sb = blob("srcs").bitcast(I32)

        groups = list(range(C))

        with (
            tc.tile_pool(name="bigp", bufs=1) as bigp,
            tc.tile_pool(name="pbp", bufs=1) as pbp,
            tc.tile_pool(name="dramp", bufs=1, space="DRAM") as dramp,
            tc.tile_pool(name="gpsum", bufs=4, space="PSUM") as gpsum,
            tc.tile_pool(name="apsum", bufs=4, space="PSUM") as apsum,
            tc.tile_pool(name="stage", bufs=4) as stage,
            tc.tile_pool(name="msgp", bufs=3) as msgp,
            tc.tile_pool(name="sp", bufs=3) as sp,
        ):
            # DRAM scratch (tracked tiles so Tile orders GEMM->collective->gather)
            p1l = dramp.tile([SLOTS, FEAT], F32, name="p1l", tag="p1l")
            p2l = dramp.tile([SLOTS, FEAT], F32, name="p2l", tag="p2l")
            p1f = dramp.tile(
                [C * SLOTS, FEAT], F32, addr_space="Shared", name="p1f", tag="p1f"
            )
            p2f = dramp.tile(
                [C * SLOTS, FEAT], F32, addr_space="Shared", name="p2f", tag="p2f"
            )

            # xtw (xT ++ W1, layer 1 GEMM input) and h2 (layer 2 output)
            # share one SBUF slot: h2's first write waits for xtw's last read
            xtw_sb = bigp.tile([FEAT, SLOTS + FEAT], F32, name="xtw_sb", tag="big")
            nc.sync.dma_start(out=xtw_sb[:], in_=xtw_h[:])
            h2_sb = None

            for layer in (1, 2):
                if layer == 2:
                    h2_sb = bigp.tile([FEAT, SLOTS], F32, name="h2_sb", tag="big")
                in_sb = xtw_sb if layer == 1 else h1_sb
                Wl = xtw_sb[:, SLOTS : SLOTS + FEAT] if layer == 1 else W2_sb[:]
                bl = b1_sb if layer == 1 else b2_sb
                pl = p1l if layer == 1 else p2l
                pf = p1f if layer == 1 else p2f
                h_out = h1_sb if layer == 1 else h2_sb

                # GEMM: p = h_prev @ W  (node-major out, staged in SBUF then
                # one bulk DMA to HBM — keeps every instruction's wait fan-in
                # within walrus' per-opcode sync-wait limits)
                p_buf = pbp.tile([P, G * FEAT], F32, name=f"p_buf{layer}", tag="pbuf")
                for g in range(G):
                    ps = gpsum.tile([P, FEAT], F32, tag="gp")
                    nc.tensor.matmul(
                        out=ps[:],
                        lhsT=in_sb[:, g * P : (g + 1) * P],
                        rhs=Wl,
                        start=True,
                        stop=True,
                    )
                    nc.vector.tensor_copy(
                        out=p_buf[:, g * FEAT : (g + 1) * FEAT], in_=ps[:]
                    )
                nc.sync.dma_start(
                    out=pl[:].rearrange("(g p) f -> p g f", p=P),
                    in_=p_buf[:].rearrange("p (g f) -> p g f", f=FEAT),
                )

                # AllGather p across cores
                nc.gpsimd.collective_compute(
                    "AllGather",
                    mybir.AluOpType.bypass,
                    replica_groups=[groups],
                    ins=[pl[:]],
                    outs=[pf[:]],
                )

                # message passing per group
                for g in range(G):
                    # One indirect DMA per 128-edge tile: the HW descriptor
                    # generator emits one descriptor per dest-AP outer entry
                    # (= partition), each reading one 64-elem row at its
                    # per-partition offset. Larger batches collapse into
                    # contiguous streams and gather the wrong rows.
                    msg = msgp.tile([P, T * FEAT], F32, tag="msg")
                    for t in range(T):
                        nc.gpsimd.indirect_dma_start(
                            out=msg[:, t * FEAT : (t + 1) * FEAT],
                            out_offset=None,
                            in_=pf[:],
                            in_offset=IndirectOffsetOnAxis(
                                ap=srcs_sb[:, g * T + t : g * T + t + 1], axis=0
                            ),
                        )
                    s_oh = sp.tile([P, T * W], F32, tag="soh")
                    s_sc = sp.tile([P, T * W], F32, tag="ssc")
                    nc.vector.tensor_tensor(
                        out=s_oh[:].rearrange("p (t w) -> p t w", w=W),
                        in0=drel_sb[:, g * T : (g + 1) * T].to_broadcast([P, T, W]),
                        in1=iota_sb[:].rearrange("p (t w) -> p t w", w=W),
                        op=mybir.AluOpType.is_equal,
                    )
                    nc.vector.tensor_tensor(
                        out=s_sc[:].rearrange("p (t w) -> p t w", w=W),
                        in0=s_oh[:].rearrange("p (t w) -> p t w", w=W),
                        in1=nrm_sb[:, g * T : (g + 1) * T].to_broadcast([P, T, W]),
                        op=mybir.AluOpType.mult,
                    )
                    if debug_outputs and layer == 1 and g == 0:
                        nc.sync.dma_start(out=dbg_msg[:], in_=msg[:])
                        nc.sync.dma_start(out=dbg_s[:], in_=s_sc[:])
                    acc = apsum.tile([FEAT, P], F32, tag="acc")
                    nc.vector.memset(acc[:], 0.0)
                    # DVE copy of edge-tile 0 gives the group's first matmul
                    # a single (DVE) wait; the gather-DMA wait lands on the
                    # t=1 matmul where it is the only one (fp32 matmuls
                    # tolerate a single sync wait)
                    msg0 = stage.tile([P, FEAT], F32, tag="msg0")
                    nc.vector.tensor_copy(out=msg0[:], in_=msg[:, 0:FEAT])
                    for t in range(T):
                        nc.tensor.matmul(
                            out=acc[:, int(lo[t]) : int(lo[t]) + W],
                            lhsT=msg0[:]
                            if t == 0
                            else msg[:, t * FEAT : (t + 1) * FEAT],
                            rhs=s_sc[:, t * W : (t + 1) * W],
                            start=False,
                            stop=(t == T - 1),
                            skip_group_check=True,
                        )
                    # relu(acc + b) on DVE: keeps all of this group's PSUM
                    # producers/consumers on one semaphore for the matmuls
                    nc.vector.tensor_scalar(
                        out=h_out[:, g * P : (g + 1) * P],
                        in0=acc[:],
                        scalar1=bl[:, 0:1],
                        scalar2=0.0,
                        op0=mybir.AluOpType.add,
                        op1=mybir.AluOpType.max,
                    )

            if debug_outputs:
                nc.sync.dma_start(out=dbg_p1l[:], in_=p1l[:])
                nc.sync.dma_start(out=dbg_p1f[:], in_=p1f[:])
                nc.sync.dma_start(out=dbg_h1[:], in_=h1_sb[:])

            # layer 3: out = h2 @ W3 + b3  (node-major, staged + bulk DMA)
            o_buf = pbp.tile([P, G * FEAT], F32, name="o_buf", tag="pbuf")
            for g in range(G):
                ps = gpsum.tile([P, FEAT], F32, tag="gp")
                nc.tensor.matmul(
                    out=ps[:],
                    lhsT=h2_sb[:, g * P : (g + 1) * P],
                    rhs=W3_sb[:],
                    start=True,
                    stop=True,
                )
                nc.vector.tensor_tensor(
                    out=o_buf[:, g * FEAT : (g + 1) * FEAT],
                    in0=ps[:],
                    in1=b3b_sb[:],
                    op=mybir.AluOpType.add,
                )
            nc.sync.dma_start(
                out=out_h[:].rearrange("(g p) f -> p g f", p=P),
                in_=o_buf[:].rearrange("p (g f) -> p g f", f=FEAT),
            )
    return nc


def make_in_maps(per_core, iota, inputs, meta):
    SLOTS = meta["SLOTS"]
    off, CB = blob_layout(meta)
    W1 = np.asarray(inputs["W1"], np.float32)
    W2 = np.asarray(inputs["W2"], np.float32)
    W3 = np.asarray(inputs["W3"], np.float32)
    b1 = np.asarray(inputs["b1"], np.float32).reshape(FEAT, 1)
    b2 = np.asarray(inputs["b2"], np.float32).reshape(FEAT, 1)
    b3b = np.tile(np.asarray(inputs["b3"], np.float32)[None, :], (P, 1))

    def fill(blob, name, arr, parts):
        c, w = off[name]
        assert arr.shape == (parts, w), (name, arr.shape, (parts, w))
        blob[:parts, c : c + w] = arr

    in_maps = []
    for c in range(meta["n_cores"]):
        pc = per_core[c]
        xtw = np.zeros((FEAT, SLOTS + FEAT), np.float32)
        xtw[:, :SLOTS] = pc["xT"]
        xtw[:, SLOTS:] = W1
        blob = np.zeros((P, CB), np.float32)
        fill(blob, "b3b", b3b, P)
        fill(blob, "iota", iota, P)
        fill(blob, "drel", pc["drel"], P)
        fill(blob, "nrm", pc["nrm"], P)
        fill(blob, "srcs", pc["srcs"].view(np.float32), P)
        fill(blob, "W2", W2, FEAT)
        fill(blob, "W3", W3, FEAT)
        fill(blob, "b1", b1, FEAT)
        fill(blob, "b2", b2, FEAT)
        in_maps.append(dict(xtw=xtw, blob=blob))
    return in_maps


def assemble_output(results, meta):
    """results: list of per-core dicts with 'out' [SLOTS, 64]."""
    C, SLOTS = meta["n_cores"], meta["SLOTS"]
    allout = np.concatenate([np.asarray(results[c]["out"]) for c in range(C)], axis=0)
    return allout[meta["node_row"]]


def kernel(**inputs) -> np.ndarray:
    x = np.asarray(inputs["x"], np.float32)
    edge_index = np.asarray(inputs["edge_index"])
    per_core, iota, meta = preprocess(x, edge_index, N_CORES)
    nc = build_nc(meta)
    nc.finalize()  # runs Bacc.compile (event-sem split, reg alloc)
    in_maps = make_in_maps(per_core, iota, inputs, meta)
    trace = bool(int(os.environ.get("BASS_GCN_TRACE", "0")))
    res = run_bass_kernel_spmd(nc, in_maps, list(range(N_CORES)), trace=trace)
    if trace and res.exec_time_ns is not None:
        print(f"HW exec time: {res.exec_time_ns} ns")
        kernel.last_exec_time_ns = res.exec_time_ns
    kernel.last_results = res
    return assemble_output(res.results, meta).astype(np.float32)


kernel.last_exec_time_ns = None
kernel.last_results = None
